# revision 39
# baseline (speedup 1.0000x reference)
"""Trainium2 Bass kernel for nn_SAW_53395033424216 (grouped-covariance loss).

Math (see reference): for each sample b and channel-group g (16 channels),
  cov[b,g] = (Xg Xg^T)/(HW-1) with Xg rows scaled by wgh; loss is the
  mean-over-B sum-over-g of the masked (strict upper triangle) abs-sum of
  cov / num_off.

Strategy:
  * Host: compute perm/wgh from classifier_w (tiny), permute channels so each
    group is 16 consecutive channels, transpose each sample to [HW, 512] and
    cast to bf16 (the 61k-entry abs-sum averages away bf16 noise; measured
    rel-err ~2.4e-6 on the fixed seed inputs).
  * Device (8 cores, 2 samples each): stream [128hw x 512ch] fp8 tiles;
    for each 128-channel block (= 8 whole groups) accumulate the 128x128
    Gram matrix over all 16384 hw rows via fp8 DoubleRow PE matmuls (each
    instruction consumes a PAIR of 128-row chunks at 0.5 cycles/row).
    Weight-scaling is bilinear -> folded into a per-block [128,128]
    mask/weight tile applied once at the end (DVE), followed by an abs
    row-reduce. Output: [128,2] per-sample partial sums per core.
  * Host: sum partials -> loss (clamp is a no-op since summands are >= 0).
"""

import os

# Whole-tile dependency tracking only: with per-subtile releases the slab DMA
# accumulates more sync-waits than the DMA pseudo-instruction format allows
# ("Too many sync wait commands" in walrus codegen).
os.environ.setdefault("BY_DEFAULT_DISABLE_SUBTILE_DEPS", "1")

import numpy as np
import ml_dtypes

import concourse.bass as bass
import concourse.mybir as mybir
from concourse.tile import TileContext
from concourse.bass_utils import run_bass_kernel_spmd

# Problem constants (hardcoded per the harness contract)
B = 16          # batch
CH = 512        # channels
H = W = 128
HW = H * W      # 16384
C = 16          # selected classes = group width
G = CH // C     # 32 groups
N_CORES = 8
SAMPLES_PER_CORE = B // N_CORES  # 2
NUM_OFF = C * (C - 1) // 2       # 120

# Data dtype on the wire/PE: bfloat16 (rel err ~2.4e-6) or float8_e4m3
# (rel err ~8.1e-4, half the DMA traffic).
DATA_DT_NAME = "float8e4"
SLAB = 8        # hw-chunks per DMA (per-partition contiguous run = SLAB*CH bytes)
N_WARMUP_MM = int(os.environ.get("KBENCH_WARMUP", "24"))  # dummy matmuls; preload keeps PE fed right after, so the HAM ramp continues on real work
N_CHUNKS = HW // 128             # 128
N_SLABS = N_CHUNKS // SLAB       # 16
N_CB = CH // 128                 # 4 channel blocks

_PROGRAM = None
LAST_RESULTS = None  # BassKernelResults of the most recent run (for test.py)


def _ensure_ntff_hook():
    """Provide antenv.axon_hooks if the image lacks it, so BASS_TRACE=1
    profiling works under axon (drives NTFF capture via the axon PJRT .so)."""
    try:
        import antenv.axon_hooks  # noqa: F401

        return
    except ImportError:
        pass
    import contextlib
    import ctypes
    import sys
    import types

    try:
        import antenv
    except ImportError:
        return

    so_path = "/opt/axon/libaxon_pjrt.so"
    if not os.path.exists(so_path):
        return
    lib = ctypes.CDLL(so_path)
    if not hasattr(lib, "axon_start_nrt_profile"):
        hook = None
    else:
        lib.axon_start_nrt_profile.argtypes = [
            ctypes.POINTER(ctypes.c_int64),
            ctypes.c_size_t,
        ]
        lib.axon_start_nrt_profile.restype = ctypes.c_int64
        lib.axon_stop_nrt_profile.argtypes = [ctypes.c_char_p]
        lib.axon_stop_nrt_profile.restype = ctypes.c_int64

        @contextlib.contextmanager
        def hook(output_dir, device_ids):
            import jax

            jax.devices()  # ensure the PJRT client exists before start
            if device_ids:
                ids = (ctypes.c_int64 * len(device_ids))(*device_ids)
                rc = lib.axon_start_nrt_profile(ids, len(device_ids))
            else:
                rc = lib.axon_start_nrt_profile(None, 0)
            if rc != 0:
                raise RuntimeError(f"axon_start_nrt_profile rc={rc}")
            try:
                yield
            finally:
                n = lib.axon_stop_nrt_profile(str(output_dir).encode())
                if n < 0:
                    raise RuntimeError(f"axon_stop_nrt_profile rc={n}")

    state = {"hook": hook}
    mod = types.ModuleType("antenv.axon_hooks")
    mod.get_axon_ntff_profile_hook = lambda: state["hook"]
    mod.set_axon_ntff_profile_hook = lambda h: state.update(hook=h)
    sys.modules["antenv.axon_hooks"] = mod
    antenv.axon_hooks = mod


_ensure_ntff_hook()


PRELOAD_SLABS = int(os.environ.get("KBENCH_PRELOAD", "2"))


def _build_program():
    nc = bass.Bass()
    f32 = mybir.dt.float32
    data_dt = getattr(mybir.dt, DATA_DT_NAME)

    # Host pre-tiled layout: [s, slab, partition, k, c] so each partition's
    # slab slice is one contiguous 8 KiB run in DRAM (max DMA efficiency).
    xt = nc.dram_tensor(
        "xt", [SAMPLES_PER_CORE, N_SLABS, 128, SLAB, CH], data_dt, kind="ExternalInput"
    )
    wm = nc.dram_tensor("wm", [N_CB, 128, 128], f32, kind="ExternalInput")
    out = nc.dram_tensor("out", [128, SAMPLES_PER_CORE], f32, kind="ExternalOutput")

    # Preload: the Tile entry sequence (ordering-mode/reg-init/staggered
    # barrier) costs ~1.7us on the SP queue before its first in-Tile DMA can
    # issue.  Fetch the first slabs of sample 0 with raw pre-Tile DMAs so the
    # DMA bus starts during that window.  Completion is signalled on a manual
    # semaphore (+16 per DMA, mirroring the HWDGE convention); PE waits on it
    # before the first preload-consuming matmul.  Writes to pre_t happen-
    # before all reads via that sem, so Tile needs no dep tracking for it.
    pre_ctx = None
    pre_t = None
    pre_sem = None
    if PRELOAD_SLABS:
        from contextlib import ExitStack

        pre_ctx = ExitStack()
        pre_t = pre_ctx.enter_context(
            nc.sbuf_tensor("preload", [128, PRELOAD_SLABS * SLAB, CH], data_dt)
        )
        pre_sem = nc.alloc_semaphore(name="preload_sem")  # held for program life
        for i in range(PRELOAD_SLABS):
            bi = nc.sync.dma_start(
                out=pre_t[:, i * SLAB : (i + 1) * SLAB, :], in_=xt[0, i]
            )
            bi.ins.sync_info = mybir.SyncInfo(
                on_wait=[],
                on_update=[
                    mybir.SyncUpdate(
                        sync_type="semaphore",
                        id=pre_sem.num,
                        update_mode="sem-add-imm",
                        ant_name=pre_sem.name,
                        update_value=16,
                    )
                ],
            )

    with TileContext(nc) as tc:
        with (
            tc.tile_pool(name="wpool", bufs=1) as wpool,
            tc.tile_pool(name="data", bufs=16) as dpool,
            tc.tile_pool(name="scratch", bufs=2) as spool,
            tc.tile_pool(name="redp", bufs=1) as redp,
            tc.tile_pool(name="psum", bufs=2, space="PSUM") as psum_pool,
        ):
            # wm load goes on the Activation queue so the SP queue's first
            # instruction is the first data slab (DMA engines start earliest).
            wm_t = wpool.tile([128, N_CB, 128], f32)
            nc.scalar.dma_start(out=wm_t, in_=wm.transpose([1, 0, 2]))

            red_all = redp.tile([128, SAMPLES_PER_CORE], f32)

            # PE warm-up: ~3us of throwaway matmuls while the first data slab
            # is still in flight, so the HAM clock gate reaches 8/8 before the
            # real stream begins.  Shares the gram0 slot tag; real use of that
            # slot starts with start=True which clears it.
            warm_in = wpool.tile([128, 128], data_dt, name="warm_in")
            nc.vector.memset(warm_in, 1)
            warm_ps = psum_pool.tile(
                [128, N_CB, 512], f32, name="warm_ps", tag="gram"
            )
            for _ in range(N_WARMUP_MM):
                nc.tensor.matmul(
                    warm_ps[:, 0, 0:128],
                    lhsT=warm_in,
                    rhs=warm_in,
                    start=True,
                    stop=True,
                )

            for s in range(SAMPLES_PER_CORE):
                # One PSUM bank per channel-block Gram: a matmul's start=True
                # clears has_written for its WHOLE bank, so interleaved
                # accumulation groups must not share a bank.  Pad the per-cb
                # stride to 512 f32 (= one full bank); only cols 0:128 are used.
                gram = psum_pool.tile([128, N_CB, 512], f32, name="gram", tag="gram")
                # The very last slab of the last sample is split (4,4): each
                # DMA completion is only visible to PE ~900ns after the bytes
                # land (sem propagation), so the optimal tail is two
                # half-slabs -- the first half's matmuls run while the second
                # half's sem is in flight.
                if s == SAMPLES_PER_CORE - 1:
                    slab_plan = [(sl * SLAB, SLAB) for sl in range(N_SLABS - 1)]
                    c_last = (N_SLABS - 1) * SLAB
                    half = SLAB // 2
                    slab_plan += [(c_last, half), (c_last + half, SLAB - half)]
                else:
                    slab_plan = [(sl * SLAB, SLAB) for sl in range(N_SLABS)]
                for c0, csz in slab_plan:
                    if s == 0 and c0 + csz <= PRELOAD_SLABS * SLAB:
                        # Slab was fetched by the pre-Tile preload DMAs.
                        dt_t = pre_t[:, c0 : c0 + csz, :]
                    else:
                        dt_full = dpool.tile([128, SLAB, CH], data_dt)
                        src_ap = xt[s, c0 // SLAB]
                        if csz != SLAB:
                            src_ap = src_ap[:, c0 % SLAB : c0 % SLAB + csz]
                        nc.sync.dma_start(out=dt_full[:, :csz], in_=src_ap)
                        dt_t = dt_full
                    # DoubleRow fp8: each matmul consumes a PAIR of hw-chunks
                    # laid out as [128p, 2, 128ch]; PE computes
                    # A^T A + B^T B in one pass at 0.5 cycles/row.
                    for k in range(0, csz, 2):
                        h = c0 + k
                        for cb in range(N_CB):
                            t2 = dt_t[:, k : k + 2, cb * 128 : (cb + 1) * 128]
                            nc.tensor.matmul(
                                gram[:, cb, 0:128],
                                lhsT=t2,
                                rhs=t2,
                                perf_mode=mybir.MatmulPerfMode.DoubleRow,
                                start=(h == 0),
                                stop=(h == N_CHUNKS - 2),
                            )
                # post-process: red[:, s] = sum_{cb,j} |gram[i,cb,j]| * wm[cb,i,j]
                scr = spool.tile([128, N_CB, 128], f32)
                nc.vector.tensor_mul(scr, gram[:, :, 0:128], wm_t[:, :, :])
                nc.vector.tensor_reduce(
                    out=red_all[:, s : s + 1],
                    in_=scr,
                    axis=mybir.AxisListType.XY,
                    op=mybir.AluOpType.add,
                    apply_absolute_value=True,
                )

            # Ship per-sample partials; the cross-sample + cross-partition sum
            # happens on the host (256 floats) to keep the device tail short.
            # Issued from the (by now idle) SP queue: HWDGE beats the gpsimd
            # SWDGE path by ~450ns of fixed overhead and skips the slow
            # gpsimd drain on the completion path.
            nc.sync.dma_start(out=out[:, :], in_=red_all, single_packet=True)

    if pre_ctx is not None:
        pre_ctx.close()
    _reduce_sync_waits(nc)
    if PRELOAD_SLABS:
        _add_preload_wait(nc, pre_sem, 16 * PRELOAD_SLABS)
    return nc


def _add_preload_wait(nc, pre_sem, value):
    """Order the PE stream after the pre-Tile preload DMAs.

    Added AFTER Tile scheduling: Tile's deadlock-checking sim only models the
    in-Tile block, where the preload sem never advances, so the wait must not
    be visible to it.  PE executes in order, so the wait may sit on any PE
    instruction at-or-before the first preload-consuming one; walrus allows
    only one wait per compute instruction, so pick the latest wait-free PE
    instruction at-or-before the first preload reference."""
    insts = [i for fn in nc.m.functions for blk in fn.blocks for i in blk.instructions]

    def refs_preload(inst):
        for a in list(inst.ins) + list(inst.outs):
            try:
                if "preload" in a.bass_ap.tensor.name:
                    return True
            except AttributeError:
                pass
        return False

    pe_insts = [
        i
        for i in insts
        if type(i).__name__ in ("InstLdweights", "InstMatmult")
    ]
    first_use = next(k for k, i in enumerate(pe_insts) if refs_preload(i))
    target = None
    for i in reversed(pe_insts[: first_use + 1]):
        if i.sync_info is None or len(i.sync_info.on_wait) == 0:
            target = i
            break
    assert target is not None, "no wait-free PE instruction before preload use"
    w = mybir.SyncWait(
        sync_type="semaphore",
        id=pre_sem.num,
        wait_mode="sem-ge-imm",
        ant_name=pre_sem.name,
        wait_value=value,
    )
    if target.sync_info is None:
        target.sync_info = mybir.SyncInfo(on_wait=[w], on_update=[])
    else:
        target.sync_info.on_wait = [w]


# Procs whose semaphores advance in instruction (program) order.  DMA lanes
# qualify: each lane's DMAs go through the same FIFO ring and complete (inc
# their lane sem) in issue order per SDMA engine.  GpSimd (Pool) does not
# (8 independent Q7 FIFOs) - we never emit Pool work.
_INORDER = ("PE", "DVE", "Activation", "SP", "DMAHW", "DMASW")


def _reduce_sync_waits(nc):
    """Walrus' per-instruction sync-wait capacity is 1 for DMA/compute
    pseudo-instructions (and small for Drain), but Tile's semaphore pass is
    not transitively minimal and can emit more.  Each carried wait also
    costs ~29ns of semaphore receive overhead in HW even when already
    satisfied, so redundant waits throttle back-to-back engine streams.

    Two reductions, both exploiting that sems are monotone (a wait that
    held once holds forever) and that _INORDER procs execute/complete in
    program order:

    (1) drop any wait implied by the instruction's own stream prefix: an
        EARLIER instruction in the same in-order proc stream already
        carried a wait implying it (directly or transitively), so it is
        satisfied before this instruction starts;
    (2) reduce every remaining multi-wait list to its weakest sufficient
        single wait: keep (sem_k >= v_k) if every other wait (sem_d >= v_d)
        must hold once sem_k reaches v_k -- true if an instruction
        at-or-before tick v_k in sem_k's stream carried (transitively) a
        wait implying it.
    """
    insts = [i for fn in nc.m.functions for blk in fn.blocks for i in blk.instructions]

    def proc_of_sem(name):
        return name.rsplit("_", 1)[0]  # e.g. "DMAHW3_44" -> "DMAHW3"

    # Per proc: ordered stream of (waits, cumulative-sem-value-after).
    streams = {}
    # Per instruction id: [(proc, stream-index, sem-value-before)]
    positions = {}

    def add_to_stream(inst, proc, waits, upd):
        lst = streams.setdefault(proc, [])
        prev = lst[-1][1] if lst else 0
        positions.setdefault(id(inst), []).append((proc, len(lst), prev))
        lst.append((waits, prev + upd))

    eng_sem = {"PE": "PE", "DVE": "DVE", "ACT": "Activation", "SP": "SP"}
    for inst in insts:
        si = inst.sync_info
        waits = [(w.ant_name, w.wait_value) for w in si.on_wait] if si else []
        if type(inst).__name__ == "InstDMACopy":
            # completion updates belong to the DMA lane proc
            for u in si.on_update:
                add_to_stream(inst, proc_of_sem(u.ant_name), waits, u.update_value)
        else:
            en = str(inst.engine).split(".")[-1]
            pref = eng_sem.get(en)
            if pref is None:
                continue
            upd = 0
            if si:
                for u in si.on_update:
                    if proc_of_sem(u.ant_name) == pref:
                        upd += u.update_value
            add_to_stream(inst, pref, waits, upd)

    from functools import lru_cache

    @lru_cache(maxsize=None)
    def holds(proc, tick, sem_d, v_d, depth=4):
        """Once `proc`'s sem has reached `tick`, does sem_d >= v_d hold?

        Covered prefix: entries up to the last one whose own completion is
        certified (cumulative sem value <= tick) have issued, so their waits
        held at some past moment; sems are monotone, so they hold now.
        """
        if proc == proc_of_sem(sem_d):
            return tick >= v_d
        if depth == 0 or not proc.startswith(_INORDER):
            return False
        stream = streams.get(proc, [])
        last = -1
        prev = 0
        for i, (waits, cum) in enumerate(stream):
            if cum > tick:
                break
            if cum > prev:
                last = i  # completing instruction within budget
            prev = cum
        for waits, _cum in stream[: last + 1]:
            for (s, v) in waits:
                if s == sem_d and v >= v_d:
                    return True
                if holds(proc_of_sem(s), v, sem_d, v_d, depth - 1):
                    return True
        return False

    @lru_cache(maxsize=None)
    def prefix_implies(proc, idx, sem_d, v_d):
        """Is (sem_d >= v_d) guaranteed once the instruction at stream
        index `idx` of in-order proc `proc` starts?  True if any EARLIER
        instruction in the stream carried a wait directly implying it."""
        for waits, _cum in streams.get(proc, [])[:idx]:
            for (s, v) in waits:
                if s == sem_d and v >= v_d:
                    return True
        return False

    for inst in insts:
        tn = type(inst).__name__
        si = inst.sync_info
        if si is None or len(si.on_wait) == 0:
            continue
        # Drop waits implied by the instruction's own position in its
        # in-order stream(s): at least `v` completions of that proc precede
        # it in program order, or an earlier same-stream instruction
        # already waited for (something implying) this wait.
        own = [
            (proc, idx, prefix)
            for proc, idx, prefix in positions.get(id(inst), [])
            if proc.startswith(_INORDER)
        ]
        kept_sw = []
        for w in si.on_wait:
            wp = proc_of_sem(w.ant_name)
            if any(proc == wp and prefix >= w.wait_value for proc, idx, prefix in own):
                continue
            kept_sw.append(w)
        if len(kept_sw) <= 1:
            si.on_wait = kept_sw
            continue
        waits = [(w.ant_name, w.wait_value) for w in kept_sw]
        chosen = None
        for k, (sem_k, v_k) in enumerate(waits):
            if not proc_of_sem(sem_k).startswith(_INORDER):
                continue
            if all(
                holds(proc_of_sem(sem_k), v_k, sem_d, v_d)
                for d, (sem_d, v_d) in enumerate(waits)
                if d != k
            ):
                chosen = k
                break
        assert chosen is not None, (
            f"{inst.name} ({tn}): cannot reduce waits to 1: {waits}"
        )
        si.on_wait = [kept_sw[chosen]]


def _host_prep(x, classifier_w, sel):
    """Compute perm / per-block weight-mask and the per-core bf16 shards."""
    x = np.asarray(x)
    w = np.asarray(classifier_w).astype(np.float32)
    sel = np.asarray(sel).astype(np.int64)

    w_abs = np.abs(w)
    idx = np.argsort(-w_abs, axis=1, kind="stable")  # matches jnp.argsort (stable)
    sig = (1.0 / (1.0 + np.exp(-w_abs.astype(np.float64)))).astype(np.float32)

    idx_sel = idx[sel]               # [C, CH]
    ch_ids = idx_sel[:, :G].T        # [G, C]
    perm = ch_ids.reshape(G * C)     # output channel g*C+c <- input channel
    wgh = sig[sel[None, :], ch_ids].reshape(G * C).astype(np.float64)

    # Per-channel-block weight/mask tile, with all scalar factors folded in:
    # wm[cb, i, j] = wgh_i * wgh_j * [same 16-group, j > i] / ((HW-1)*NUM_OFF*B)
    wm = np.zeros((N_CB, 128, 128), dtype=np.float64)
    scale = 1.0 / ((HW - 1) * NUM_OFF * B)
    ii, jj = np.meshgrid(np.arange(128), np.arange(128), indexing="ij")
    blockmask = ((ii // C) == (jj // C)) & (jj > ii)
    for cb in range(N_CB):
        wloc = wgh[cb * 128 : (cb + 1) * 128]
        wm[cb] = np.outer(wloc, wloc) * blockmask * scale
    wm = wm.astype(np.float32)

    # Per-core shards: samples [2c, 2c+1] -> permuted channels, hw-major,
    # pre-tiled as [s, slab, partition, k, c] so each partition's slab row is
    # one contiguous 8 KiB DRAM run.
    xr = x.reshape(B, CH, HW)
    shards = []
    for c in range(N_CORES):
        xs = xr[c * SAMPLES_PER_CORE : (c + 1) * SAMPLES_PER_CORE][:, perm, :]
        np_dt = mybir.dt.np(getattr(mybir.dt, DATA_DT_NAME))
        xb = xs.transpose(0, 2, 1).astype(np_dt)  # [S, HW, CH]
        xt = np.ascontiguousarray(
            xb.reshape(SAMPLES_PER_CORE, N_SLABS, SLAB, 128, CH).transpose(
                0, 1, 3, 2, 4
            )
        )
        shards.append(xt)
    return shards, wm


def kernel(x, classifier_w, sel):
    global _PROGRAM, LAST_RESULTS
    assert x.shape == (B, CH, H, W), x.shape

    shards, wm = _host_prep(x, classifier_w, sel)

    if _PROGRAM is None:
        _PROGRAM = _build_program()

    in_maps = [{"xt": shards[c], "wm": wm} for c in range(N_CORES)]
    LAST_RESULTS = run_bass_kernel_spmd(_PROGRAM, in_maps, core_ids=list(range(N_CORES)))

    total = np.float64(0.0)
    for r in LAST_RESULTS.results:
        total += np.float64(r["out"].sum(dtype=np.float64))
    return np.array([total], dtype=np.float32)

